# revision 1
# baseline (speedup 1.0000x reference)
"""Causal self-attention (B=4, T=2048, D=1024, H=16) on 8 trn2 NeuronCores.

Sharding: Megatron-style tensor parallel over heads (TP=2) x data parallel
over batch (DP=4). Core c handles batch c//2 and head-group c%2 (8 heads).
Each core computes its QKV projection slice, causal attention for its 8
heads, and a partial output projection; the host sums the two TP partials
per batch and adds b_proj.

All matmuls run in fp16 (fp32 PSUM accumulation); softmax runs in fp32 on
the scalar engine (exp) / DVE (reciprocal).
"""
import sys

sys.path.insert(0, "/opt/trn_rl_repo")

import numpy as np
import ml_dtypes

import concourse.bass as bass
import concourse.tile as tile
from concourse import bacc, mybir
from concourse.bass_utils import run_bass_kernel_spmd

B, T, D, H = 4, 2048, 1024, 16
HD = 64            # head dim
HL = 8             # heads per core (TP=2)
DL = HL * HD       # 512 local qkv width
KCH = D // 128     # 8 contraction chunks for QKV
TCH = T // 128     # 16 T chunks of 128
TB = T // 512      # 4 T blocks of 512
F16 = mybir.dt.float16
F32 = mybir.dt.float32
NEG = -1.0e30

_cache = {}


def _build():
    nc = bacc.Bacc("TRN2", target_bir_lowering=False, num_devices=8)

    xT = nc.dram_tensor("xT", [D, T], F16, kind="ExternalInput")
    wqk = nc.dram_tensor("wqk", [D, 2 * DL], F16, kind="ExternalInput")
    bqk = nc.dram_tensor("bqk", [128, 2 * DL // 128], F32, kind="ExternalInput")
    wv = nc.dram_tensor("wv", [D, DL], F16, kind="ExternalInput")
    bv = nc.dram_tensor("bv", [1, DL], F32, kind="ExternalInput")
    wp = nc.dram_tensor("wp", [DL, D], F16, kind="ExternalInput")
    tri = nc.dram_tensor("tri", [128, 128], F32, kind="ExternalInput")
    out = nc.dram_tensor("out", [T, D], F32, kind="ExternalOutput")

    with tile.TileContext(nc) as tc:
        with (
            tc.tile_pool(name="const", bufs=1) as const,
            tc.tile_pool(name="acts", bufs=1) as acts,
            tc.tile_pool(name="work", bufs=4) as work,
            tc.tile_pool(name="small", bufs=3) as small,
            tc.tile_pool(name="outp", bufs=3) as outp,
            tc.tile_pool(name="psb", bufs=3, space="PSUM") as psb,
            tc.tile_pool(name="psy", bufs=2, space="PSUM") as psy,
        ):
            # ---- resident inputs ----
            xT_sb = []
            wqk_sb = []
            wv_sb = []
            for k in range(KCH):
                xt = const.tile([128, T], F16, name=f"xT{k}", tag=f"xT{k}")
                nc.sync.dma_start(out=xt[:, 0:T // 2],
                                  in_=xT[128 * k:128 * (k + 1), 0:T // 2])
                xT_sb.append(xt)
                wq = const.tile([128, 2 * DL], F16, name=f"wqk{k}", tag=f"wqk{k}")
                nc.sync.dma_start(out=wq, in_=wqk[128 * k:128 * (k + 1), :])
                wqk_sb.append(wq)
                wvt = const.tile([128, DL], F16, name=f"wv{k}", tag=f"wv{k}")
                nc.gpsimd.dma_start(out=wvt, in_=wv[128 * k:128 * (k + 1), :])
                wv_sb.append(wvt)
            for k in range(KCH):
                nc.sync.dma_start(out=xT_sb[k][:, T // 2:T],
                                  in_=xT[128 * k:128 * (k + 1), T // 2:T])
            wp_sb = []
            for c in range(DL // 128):
                wpt = const.tile([128, D], F16, name=f"wp{c}", tag=f"wp{c}")
                nc.gpsimd.dma_start(out=wpt, in_=wp[128 * c:128 * (c + 1), :])
                wp_sb.append(wpt)
            bqk_sb = const.tile([128, 2 * DL // 128], F32)
            nc.gpsimd.dma_start(out=bqk_sb, in_=bqk[:, :])
            bv_sb = const.tile([1, DL], F32)
            nc.gpsimd.dma_start(out=bv_sb, in_=bv[:, :])
            tri_sb = const.tile([128, 128], F32)
            nc.gpsimd.dma_start(out=tri_sb, in_=tri[:, :])
            bvb_sb = const.tile([128, DL], F32)
            nc.gpsimd.partition_broadcast(bvb_sb, bv_sb)

            # ---- persistent activations ----
            qT_sb = [acts.tile([128, T], F16, name=f"qT{c}", tag=f"qT{c}")
                     for c in range(4)]
            # kT stored per head, zero-padded to K=128: head 2c occupies
            # partitions 0:64 (64:128 zero), head 2c+1 partitions 64:128
            # (0:64 zero).  This keeps every S matmul full-array (no
            # row-group masking, which stops the PE activity monitor from
            # registering "busy" and parks the clock at half rate).
            kT2_sb = [acts.tile([128, T], F16, name=f"kT2h{h}", tag=f"kT2h{h}")
                      for h in range(HL)]
            for h in range(HL):
                z0, z1 = (64, 128) if h % 2 == 0 else (0, 64)
                nc.gpsimd.memset(kT2_sb[h][z0:z1, :], 0.0)
            vaug = [acts.tile([128, HL * (HD + 1)], F16, name=f"va{t}",
                              tag=f"va{t}") for t in range(TCH)]
            yT_sb = [acts.tile([128, T], F16, name=f"yT{c}", tag=f"yT{c}")
                     for c in range(4)]

            # ---- streamed pipeline over T-block pairs (tb2 = 1024 rows) ----
            # Each tb2 round: QKV projection for the block, then causal
            # attention for q0 blocks 2*tb2 and 2*tb2+1, then the output
            # projection for those rows.  Later rounds' projection matmuls
            # (PE) overlap earlier rounds' softmax exps (scalar engine).
            for tb2 in range(TB // 2):
                # -- qT / kT = (w_slice)^T @ xT for this block --
                for cc in range(2 * DL // 128):
                    ps_w = psb.tile([128, 1024], F32, name="psB", tag="psB")
                    for half in range(2):
                        tb = 2 * tb2 + half
                        for k in range(KCH):
                            nc.tensor.matmul(
                                ps_w[:, 512 * half:512 * (half + 1)],
                                wqk_sb[k][:, 128 * cc:128 * (cc + 1)],
                                xT_sb[k][:, 512 * tb:512 * (tb + 1)],
                                start=(k == 0), stop=(k == KCH - 1),
                            )
                    tbs = slice(1024 * tb2, 1024 * (tb2 + 1))
                    if cc < 4:
                        nc.vector.tensor_scalar_add(
                            out=qT_sb[cc][:, tbs],
                            in0=ps_w,
                            scalar1=bqk_sb[:, cc:cc + 1],
                        )
                    else:
                        hA = 2 * (cc - 4)
                        nc.vector.tensor_scalar_add(
                            out=kT2_sb[hA][0:64, tbs],
                            in0=ps_w[0:64, :],
                            scalar1=bqk_sb[0:64, cc:cc + 1],
                        )
                        nc.vector.tensor_scalar_add(
                            out=kT2_sb[hA + 1][64:128, tbs],
                            in0=ps_w[64:128, :],
                            scalar1=bqk_sb[64:128, cc:cc + 1],
                        )
                # -- v (natural layout) + ones column for this block --
                for t2 in range(4 * tb2, 4 * (tb2 + 1)):
                    ps_w = psb.tile([128, 1024], F32, name="psB", tag="psB")
                    for half in range(2):
                        t = 2 * t2 + half
                        hs = slice(512 * half, 512 * (half + 1))
                        for k in range(KCH):
                            nc.tensor.matmul(
                                ps_w[:, hs],
                                xT_sb[k][:, 128 * t:128 * (t + 1)],
                                wv_sb[k],
                                start=(k == 0), stop=(k == KCH - 1),
                            )
                    for half in range(2):
                        t = 2 * t2 + half
                        va = vaug[t]
                        va3 = va.rearrange("p (h c) -> p h c", c=HD + 1)
                        nc.vector.tensor_add(
                            va3[:, :, 0:HD],
                            ps_w[:, 512 * half:512 * (half + 1)].rearrange(
                                "p (h d) -> p h d", d=HD),
                            bvb_sb.rearrange("p (h d) -> p h d", d=HD),
                        )
                        nc.gpsimd.memset(va3[:, :, HD], 1.0)
                # -- attention for q0 in {2*tb2, 2*tb2+1}, all head pairs --
                for q0 in (2 * tb2, 2 * tb2 + 1):
                    for c in range(4):
                        ntiles = 4 * q0 + 4
                        ps_ys = [psy.tile([HD + 1, 512], F32, name="psY",
                                          tag="psY") for p in range(2)]
                        for t in range(ntiles):
                            m = t - 4 * q0
                            lo = 128 * m if m > 0 else 0
                            # both heads' scores side by side, 2-bank psum
                            ps_s = psb.tile([128, 1024], F32, name="psB",
                                            tag="psB")
                            for p in range(2):  # the two heads 2c, 2c+1
                                # full-K matmul: zero-padded kT kills the
                                # other head's rows of qT
                                nc.tensor.matmul(
                                    ps_s[:, 512 * p + lo:512 * (p + 1)],
                                    kT2_sb[2 * c + p][:,
                                                      128 * t:128 * (t + 1)],
                                    qT_sb[c][:,
                                             512 * q0 + lo:512 * (q0 + 1)],
                                    start=True, stop=True,
                                )
                            if m >= 0:
                                # one masked add over both heads via 3D AP
                                seg = ps_s.rearrange("p (u f) -> p u f", u=2)
                                nc.vector.tensor_add(
                                    seg[:, :, lo:lo + 128],
                                    seg[:, :, lo:lo + 128],
                                    tri_sb.unsqueeze(1).broadcast_to(
                                        [128, 2, 128]),
                                )
                            es = work.tile([128, 1024], F16, name="es",
                                           tag="es")
                            nc.scalar.activation(
                                out=es[:, lo:1024],
                                in_=ps_s[:, lo:1024],
                                func=mybir.ActivationFunctionType.Exp,
                            )
                            for p in range(2):
                                h = 2 * c + p
                                nc.tensor.matmul(
                                    ps_ys[p][:, lo:512],
                                    vaug[t][:,
                                            (HD + 1) * h:(HD + 1) * (h + 1)],
                                    es[:, 512 * p + lo:512 * (p + 1)],
                                    start=(t == 0), stop=(t == ntiles - 1),
                                )
                        for p in range(2):
                            poff = 64 * p
                            # free the PV psum quickly: copy to SBUF, then
                            # normalize off SBUF.  partition_broadcast needs
                            # its source at base partition 0 (dn copy).
                            ys = small.tile([64, 512], F32, name="ys",
                                            tag="ys")
                            nc.vector.tensor_copy(ys, ps_ys[p][0:HD, :])
                            dn = small.tile([1, 512], F32, name="dn",
                                            tag="dn")
                            nc.vector.tensor_copy(dn, ps_ys[p][HD:HD + 1, :])
                            dnb = small.tile([64, 512], F32, name="dnb",
                                             tag="dnb")
                            nc.gpsimd.partition_broadcast(dnb, dn)
                            rcb = small.tile([64, 512], F32, name="rcb",
                                             tag="rcb")
                            nc.vector.reciprocal_approx_fast(rcb, dnb)
                            nc.vector.tensor_mul(
                                yT_sb[c][poff:poff + 64,
                                         512 * q0:512 * (q0 + 1)],
                                ys,
                                rcb,
                            )
                # -- partial out projection for this block's rows --
                for t in range(8 * tb2, 8 * (tb2 + 1)):
                    ps_o = psb.tile([128, 1024], F32, name="psB", tag="psB")
                    for nb in range(D // 512):
                        for c in range(DL // 128):
                            nc.tensor.matmul(
                                ps_o[:, 512 * nb:512 * (nb + 1)],
                                yT_sb[c][:, 128 * t:128 * (t + 1)],
                                wp_sb[c][:, 512 * nb:512 * (nb + 1)],
                                start=(c == 0), stop=(c == DL // 128 - 1),
                            )
                    ob = outp.tile([128, 1024], F32, name="ob", tag="ob")
                    nc.vector.tensor_copy(ob, ps_o)
                    nc.sync.dma_start(
                        out=out[128 * t:128 * (t + 1), :],
                        in_=ob,
                    )

    nc.finalize()
    return nc


def _enable_trace_hooks():
    """Inject antenv.axon_hooks + no-op artifact upload so that
    run_bass_kernel_spmd(trace=True) works under axon in this image."""
    import types
    import antenv

    if "antenv.axon_hooks" not in sys.modules:
        mod = types.ModuleType("antenv.axon_hooks")
        state = {"hook": None}
        mod.set_axon_ntff_profile_hook = lambda h: state.__setitem__("hook", h)
        mod.get_axon_ntff_profile_hook = lambda: state["hook"]
        sys.modules["antenv.axon_hooks"] = mod
        antenv.axon_hooks = mod
        from trn_agent_boot.trn_boot import _ntff_profile_via_ctypes

        mod.set_axon_ntff_profile_hook(
            _ntff_profile_via_ctypes("/opt/axon/libaxon_pjrt.so"))
    from concourse import bass_utils as bu

    bu.upload_artifacts = lambda tmpdir: str(tmpdir)


def kernel(x, w_attn, b_attn, w_proj, b_proj, _trace=False):
    x = np.asarray(x)
    w_attn = np.asarray(w_attn)
    b_attn = np.asarray(b_attn)
    w_proj = np.asarray(w_proj)
    b_proj = np.asarray(b_proj)

    if "nc" not in _cache:
        _cache["nc"] = _build()
    nc = _cache["nc"]

    scale = 1.0 / np.sqrt(HD)
    f16 = ml_dtypes.float16 if not hasattr(np, "float16") else np.float16
    tri = np.where(np.arange(128)[:, None] <= np.arange(128)[None, :],
                   np.float32(0.0), np.float32(NEG)).astype(np.float32)

    in_maps = []
    for core in range(8):
        b, hg = core // 2, core % 2
        qs = slice(hg * DL, (hg + 1) * DL)
        ks = slice(D + hg * DL, D + (hg + 1) * DL)
        vs = slice(2 * D + hg * DL, 2 * D + (hg + 1) * DL)
        wq = (w_attn[:, qs] * scale).astype(f16)
        wk = w_attn[:, ks].astype(f16)
        wqk_host = np.concatenate([wq, wk], axis=1)
        bqk_host = np.concatenate(
            [b_attn[qs] * scale, b_attn[ks]]).astype(np.float32)
        in_maps.append({
            "xT": np.ascontiguousarray(x[b].T).astype(f16),
            "wqk": np.ascontiguousarray(wqk_host),
            "bqk": np.ascontiguousarray(bqk_host.reshape(8, 128).T),
            "wv": np.ascontiguousarray(w_attn[:, vs]).astype(f16),
            "bv": np.ascontiguousarray(b_attn[vs][None, :]).astype(np.float32),
            "wp": np.ascontiguousarray(w_proj[hg * DL:(hg + 1) * DL, :]).astype(f16),
            "tri": tri,
        })

    kwargs = {}
    if _trace:
        _enable_trace_hooks()
        kwargs = dict(trace=True, trace_cores=[0])
    res = run_bass_kernel_spmd(nc, in_maps, core_ids=list(range(8)), **kwargs)

    outp = np.empty((B, T, D), np.float32)
    for b in range(B):
        outp[b] = res.results[2 * b]["out"] + res.results[2 * b + 1]["out"]
    outp += b_proj.astype(np.float32)

    if _trace:
        print(f"HW exec time: {res.exec_time_ns} ns")
    return outp



# revision 2
# speedup vs baseline: 1.0454x; 1.0454x over previous
"""Causal self-attention (B=4, T=2048, D=1024, H=16) on 8 trn2 NeuronCores.

Sharding: Megatron-style tensor parallel over heads (TP=2) x data parallel
over batch (DP=4). Core c handles batch c//2 and head-group c%2 (8 heads).
Each core computes its QKV projection slice, causal attention for its 8
heads, and a partial output projection; the host sums the two TP partials
per batch and adds b_proj (plus the folded v-bias contribution).

Schedule: single software-pipelined stream.  Attention for query block q0
is exp-rate-limited on the scalar engine, so independent projection
matmuls (QKV of block q0+1, output projection of earlier blocks) are
interleaved into the attention instruction stream as PE filler.  The v
bias is folded into a host-side output correction (attn rows sum to 1),
and the output partials are written as fp16 to halve the drain DMA.
"""
import sys
from collections import deque

sys.path.insert(0, "/opt/trn_rl_repo")

import numpy as np
import ml_dtypes

import concourse.bass as bass
import concourse.tile as tile
from concourse import bacc, mybir
from concourse.bass_utils import run_bass_kernel_spmd

B, T, D, H = 4, 2048, 1024, 16
HD = 64            # head dim
HL = 8             # heads per core (TP=2)
DL = HL * HD       # 512 local qkv width
KCH = D // 128     # 8 contraction chunks
TCH = T // 128     # 16 token tiles of 128
NB = T // 512      # 4 query blocks of 512
F16 = mybir.dt.float16
F32 = mybir.dt.float32
NEG = -1.0e30

_cache = {}


def _build():
    nc = bacc.Bacc("TRN2", target_bir_lowering=False, num_devices=8)

    xT = nc.dram_tensor("xT", [D, T], F16, kind="ExternalInput")
    wqk = nc.dram_tensor("wqk", [D, 2 * DL], F16, kind="ExternalInput")
    bqk = nc.dram_tensor("bqk", [128, 2 * DL // 128], F32, kind="ExternalInput")
    wv = nc.dram_tensor("wv", [D, DL], F16, kind="ExternalInput")
    wp = nc.dram_tensor("wp", [DL, D], F16, kind="ExternalInput")
    tri = nc.dram_tensor("tri", [128, 128], F32, kind="ExternalInput")
    out = nc.dram_tensor("out", [T, D], F16, kind="ExternalOutput")

    with tile.TileContext(nc) as tc:
        with (
            tc.tile_pool(name="const", bufs=1) as const,
            tc.tile_pool(name="acts", bufs=1) as acts,
            tc.tile_pool(name="work", bufs=4) as work,
            tc.tile_pool(name="small", bufs=4) as small,
            tc.tile_pool(name="outp", bufs=3) as outp,
            tc.tile_pool(name="psS", bufs=2, space="PSUM") as psS,
            tc.tile_pool(name="psY", bufs=2, space="PSUM") as psY,
            tc.tile_pool(name="psF", bufs=2, space="PSUM") as psF,
        ):
            # ---- SBUF residents ----
            xT_sb = [const.tile([128, T], F16, name=f"xT{k}", tag=f"xT{k}")
                     for k in range(KCH)]
            wqk_sb = [const.tile([128, 2 * DL], F16, name=f"wqk{k}",
                                 tag=f"wqk{k}") for k in range(KCH)]
            wv_sb = [const.tile([128, DL], F16, name=f"wv{k}", tag=f"wv{k}")
                     for k in range(KCH)]
            wp_sb = [const.tile([128, D], F16, name=f"wp{c}", tag=f"wp{c}")
                     for c in range(DL // 128)]
            bqk_sb = const.tile([128, 2 * DL // 128], F32)
            tri_sb = const.tile([128, 128], F32)

            qT_sb = [acts.tile([128, T], F16, name=f"qT{c}", tag=f"qT{c}")
                     for c in range(4)]
            # kT per head, zero-padded to K=128: head 2c in partitions 0:64
            # of kT2[2c], head 2c+1 in partitions 64:128 of kT2[2c+1]; the
            # other half stays zero so S matmuls run full-K (no row-group
            # masking -> PE clock stays at full rate).
            kT2_sb = [acts.tile([128, T], F16, name=f"kT2h{h}", tag=f"kT2h{h}")
                      for h in range(HL)]
            vaug = [acts.tile([128, HL * (HD + 1)], F16, name=f"va{t}",
                              tag=f"va{t}") for t in range(TCH)]
            yT_sb = [acts.tile([128, T], F16, name=f"yT{c}", tag=f"yT{c}")
                     for c in range(4)]

            # zero the unused kT halves before any S matmul reads them
            for h in range(HL):
                z0, z1 = (64, 128) if h % 2 == 0 else (0, 64)
                nc.gpsimd.memset(kT2_sb[h][z0:z1, :], 0.0)

            # ---- input DMAs, consumption order: block-0 x + wqk first ----
            for k in range(KCH):
                nc.sync.dma_start(out=xT_sb[k][:, 0:512],
                                  in_=xT[128 * k:128 * (k + 1), 0:512])
                nc.sync.dma_start(out=wqk_sb[k],
                                  in_=wqk[128 * k:128 * (k + 1), :])
            for k in range(KCH):
                nc.gpsimd.dma_start(out=wv_sb[k],
                                    in_=wv[128 * k:128 * (k + 1), :])
            nc.gpsimd.dma_start(out=bqk_sb, in_=bqk[:, :])
            nc.gpsimd.dma_start(out=tri_sb, in_=tri[:, :])
            for b in range(1, NB):
                for k in range(KCH):
                    nc.sync.dma_start(
                        out=xT_sb[k][:, 512 * b:512 * (b + 1)],
                        in_=xT[128 * k:128 * (k + 1), 512 * b:512 * (b + 1)])
            for c in range(DL // 128):
                nc.gpsimd.dma_start(out=wp_sb[c],
                                    in_=wp[128 * c:128 * (c + 1), :])

            # ---- filler step machinery ----
            fill = deque()

            def qk_chain_steps(cc, b):
                st = {}
                steps = []
                for k in range(KCH):
                    def mm(k=k, cc=cc, b=b):
                        if k == 0:
                            st["ps"] = psF.tile([128, 512], F32, name="psF",
                                                tag="psF")
                        nc.tensor.matmul(
                            st["ps"],
                            wqk_sb[k][:, 128 * cc:128 * (cc + 1)],
                            xT_sb[k][:, 512 * b:512 * (b + 1)],
                            start=(k == 0), stop=(k == KCH - 1))
                    steps.append(mm)

                def bias(cc=cc, b=b):
                    ps = st["ps"]
                    tbs = slice(512 * b, 512 * (b + 1))
                    if cc < 4:
                        nc.vector.tensor_scalar_add(
                            out=qT_sb[cc][:, tbs], in0=ps,
                            scalar1=bqk_sb[:, cc:cc + 1])
                    else:
                        hA = 2 * (cc - 4)
                        nc.vector.tensor_scalar_add(
                            out=kT2_sb[hA][0:64, tbs], in0=ps[0:64, :],
                            scalar1=bqk_sb[0:64, cc:cc + 1])
                        nc.vector.tensor_scalar_add(
                            out=kT2_sb[hA + 1][64:128, tbs],
                            in0=ps[64:128, :],
                            scalar1=bqk_sb[64:128, cc:cc + 1])
                steps.append(bias)
                return steps

            def v_chain_steps(t):
                st = {}
                steps = []
                for k in range(KCH):
                    def mm(k=k, t=t):
                        if k == 0:
                            st["ps"] = psF.tile([128, 512], F32, name="psF",
                                                tag="psF")
                        nc.tensor.matmul(
                            st["ps"],
                            xT_sb[k][:, 128 * t:128 * (t + 1)],
                            wv_sb[k],
                            start=(k == 0), stop=(k == KCH - 1))
                    steps.append(mm)

                def cast(t=t):
                    va3 = vaug[t].rearrange("p (h c) -> p h c", c=HD + 1)
                    nc.vector.tensor_copy(
                        va3[:, :, 0:HD],
                        st["ps"].rearrange("p (h d) -> p h d", d=HD))
                    nc.gpsimd.memset(va3[:, :, HD], 1.0)
                steps.append(cast)
                return steps

            def oproj_steps(t, half):
                st = {}
                steps = []
                for c in range(DL // 128):
                    def mm(c=c, t=t, half=half):
                        if c == 0:
                            st["ps"] = psF.tile([128, 512], F32, name="psF",
                                                tag="psF")
                        nc.tensor.matmul(
                            st["ps"],
                            yT_sb[c][:, 128 * t:128 * (t + 1)],
                            wp_sb[c][:, 512 * half:512 * (half + 1)],
                            start=(c == 0), stop=(c == DL // 128 - 1))
                    steps.append(mm)

                def castdma(t=t, half=half):
                    oc = outp.tile([128, 512], F16, name="oc", tag="oc")
                    nc.vector.tensor_copy(oc, st["ps"])
                    nc.sync.dma_start(
                        out=out[128 * t:128 * (t + 1),
                                512 * half:512 * (half + 1)],
                        in_=oc)
                steps.append(castdma)
                return steps

            def qkv_block_steps(b):
                steps = []
                order = [0, 4]
                steps += qk_chain_steps(0, b)
                steps += qk_chain_steps(4, b)
                for t in range(4 * b, 4 * b + 4):
                    steps += v_chain_steps(t)
                for cc in (1, 5, 2, 6, 3, 7):
                    steps += qk_chain_steps(cc, b)
                return steps

            def oproj_block_steps(q0):
                steps = []
                for t in range(4 * q0, 4 * q0 + 4):
                    for half in range(2):
                        steps += oproj_steps(t, half)
                return steps

            # ---- QKV for block 0: emitted directly ----
            for st in qkv_block_steps(0):
                st()

            # ---- attention pipeline over query blocks ----
            for q0 in range(NB):
                if q0 + 1 < NB:
                    fill.extend(qkv_block_steps(q0 + 1))
                if q0 == NB - 1:
                    for qq in range(NB - 1):
                        fill.extend(oproj_block_steps(qq))

                ntiles = 4 * q0 + 4
                tiles_total = 4 * ntiles
                tiles_done = 0
                for c in range(4):
                    ps_ys = [psY.tile([HD + 1, 512], F32, name="psY",
                                      tag="psY") for p in range(2)]
                    es_prev = None

                    def emit_PV(t, es):
                        m = t - 4 * q0
                        lo = 128 * m if m > 0 else 0
                        for p in range(2):
                            nc.tensor.matmul(
                                ps_ys[p][:, lo:512],
                                vaug[t][:, (HD + 1) * (2 * c + p):
                                        (HD + 1) * (2 * c + p + 1)],
                                es[:, 512 * p + lo:512 * (p + 1)],
                                start=(t == 0), stop=(t == ntiles - 1))

                    for t in range(ntiles):
                        # filler first so a stalled S doesn't block it
                        left = tiles_total - tiles_done
                        pace = (len(fill) + left - 1) // left if fill else 0
                        for _ in range(min(pace, 6)):
                            if fill:
                                fill.popleft()()
                        tiles_done += 1

                        m = t - 4 * q0
                        lo = 128 * m if m > 0 else 0
                        ps_s = psS.tile([128, 1024], F32, name="psS",
                                        tag="psS")
                        for p in range(2):
                            nc.tensor.matmul(
                                ps_s[:, 512 * p + lo:512 * (p + 1)],
                                kT2_sb[2 * c + p][:, 128 * t:128 * (t + 1)],
                                qT_sb[c][:, 512 * q0 + lo:512 * (q0 + 1)],
                                start=True, stop=True)
                        if m >= 0:
                            seg = ps_s.rearrange("p (u f) -> p u f", u=2)
                            nc.vector.tensor_add(
                                seg[:, :, lo:lo + 128],
                                seg[:, :, lo:lo + 128],
                                tri_sb.unsqueeze(1).broadcast_to(
                                    [128, 2, 128]))
                        es = work.tile([128, 1024], F16, name="es", tag="es")
                        nc.scalar.activation(
                            out=es[:, lo:1024], in_=ps_s[:, lo:1024],
                            func=mybir.ActivationFunctionType.Exp)
                        if es_prev is not None:
                            emit_PV(t - 1, es_prev)
                        es_prev = es
                    emit_PV(ntiles - 1, es_prev)

                    # normalize: y = yhat / denom (denom = ones-row of PV)
                    for p in range(2):
                        dn = small.tile([1, 512], F32, name="dn", tag="dn")
                        nc.vector.tensor_copy(dn, ps_ys[p][HD:HD + 1, :])
                        dnb = small.tile([64, 512], F32, name="dnb",
                                         tag="dnb")
                        nc.gpsimd.partition_broadcast(dnb, dn)
                        rcb = small.tile([64, 512], F32, name="rcb",
                                         tag="rcb")
                        nc.vector.reciprocal_approx_fast(rcb, dnb)
                        nc.vector.tensor_mul(
                            yT_sb[c][64 * p:64 * (p + 1),
                                     512 * q0:512 * (q0 + 1)],
                            ps_ys[p][0:HD, :],
                            rcb)

            # drain leftover filler, then the final block's projection
            while fill:
                fill.popleft()()
            for st in oproj_block_steps(NB - 1):
                st()

    nc.finalize()
    return nc


def _enable_trace_hooks():
    """Inject antenv.axon_hooks + no-op artifact upload so that
    run_bass_kernel_spmd(trace=True) works under axon in this image."""
    import types
    import antenv

    if "antenv.axon_hooks" not in sys.modules:
        mod = types.ModuleType("antenv.axon_hooks")
        state = {"hook": None}
        mod.set_axon_ntff_profile_hook = lambda h: state.__setitem__("hook", h)
        mod.get_axon_ntff_profile_hook = lambda: state["hook"]
        sys.modules["antenv.axon_hooks"] = mod
        antenv.axon_hooks = mod
        from trn_agent_boot.trn_boot import _ntff_profile_via_ctypes

        mod.set_axon_ntff_profile_hook(
            _ntff_profile_via_ctypes("/opt/axon/libaxon_pjrt.so"))
    from concourse import bass_utils as bu

    bu.upload_artifacts = lambda tmpdir: str(tmpdir)


def kernel(x, w_attn, b_attn, w_proj, b_proj, _trace=False):
    x = np.asarray(x)
    w_attn = np.asarray(w_attn)
    b_attn = np.asarray(b_attn)
    w_proj = np.asarray(w_proj)
    b_proj = np.asarray(b_proj)

    if "nc" not in _cache:
        _cache["nc"] = _build()
    nc = _cache["nc"]

    scale = 1.0 / np.sqrt(HD)
    f16 = np.float16
    tri = np.where(np.arange(128)[:, None] <= np.arange(128)[None, :],
                   np.float32(0.0), np.float32(NEG)).astype(np.float32)

    in_maps = []
    for core in range(8):
        b, hg = core // 2, core % 2
        qs = slice(hg * DL, (hg + 1) * DL)
        ks = slice(D + hg * DL, D + (hg + 1) * DL)
        wq = (w_attn[:, qs] * scale).astype(f16)
        wk = w_attn[:, ks].astype(f16)
        wqk_host = np.concatenate([wq, wk], axis=1)
        bqk_host = np.concatenate(
            [b_attn[qs] * scale, b_attn[ks]]).astype(np.float32)
        vs = slice(2 * D + hg * DL, 2 * D + (hg + 1) * DL)
        in_maps.append({
            "xT": np.ascontiguousarray(x[b].T).astype(f16),
            "wqk": np.ascontiguousarray(wqk_host),
            "bqk": np.ascontiguousarray(bqk_host.reshape(8, 128).T),
            "wv": np.ascontiguousarray(w_attn[:, vs]).astype(f16),
            "wp": np.ascontiguousarray(
                w_proj[hg * DL:(hg + 1) * DL, :]).astype(f16),
            "tri": tri,
        })

    kwargs = {}
    if _trace:
        _enable_trace_hooks()
        kwargs = dict(trace=True, trace_cores=[0])
    res = run_bass_kernel_spmd(nc, in_maps, core_ids=list(range(8)), **kwargs)

    # host epilogue: sum TP partials, add b_proj and the folded v-bias term
    bias_total = (b_attn[2 * D:].astype(np.float32) @
                  w_proj.astype(np.float32)) + b_proj.astype(np.float32)
    outp = np.empty((B, T, D), np.float32)
    for b in range(B):
        outp[b] = (res.results[2 * b]["out"].astype(np.float32) +
                   res.results[2 * b + 1]["out"].astype(np.float32))
    outp += bias_total

    if _trace:
        print(f"HW exec time: {res.exec_time_ns} ns")
    return outp


# revision 7
# speedup vs baseline: 1.1008x; 1.0530x over previous
"""Causal self-attention (B=4, T=2048, D=1024, H=16) on 8 trn2 NeuronCores.

Sharding: Megatron-style tensor parallel over heads (TP=2) x data parallel
over batch (DP=4). Core c handles batch c//2 and head-group c%2 (8 heads).
Each core computes its QKV projection slice, causal attention for its 8
heads, and a partial output projection; the host sums the two TP partials
per batch and adds b_proj (plus the folded v-bias contribution).

Schedule: single software-pipelined stream.  Attention for query block q0
is exp-rate-limited on the scalar engine, so independent projection
matmuls (QKV of block q0+1, output projection of earlier blocks) are
interleaved into the attention instruction stream as PE filler.  The v
bias is folded into a host-side output correction (attn rows sum to 1),
and the output partials are written as fp16 to halve the drain DMA.
"""
import sys
from collections import deque

sys.path.insert(0, "/opt/trn_rl_repo")

import numpy as np
import ml_dtypes

import concourse.bass as bass
import concourse.tile as tile
from concourse import bacc, mybir
from concourse.bass_utils import run_bass_kernel_spmd

B, T, D, H = 4, 2048, 1024, 16
HD = 64            # head dim
HL = 8             # heads per core (TP=2)
DL = HL * HD       # 512 local qkv width
KCH = D // 128     # 8 contraction chunks
TCH = T // 128     # 16 token tiles of 128
NB = T // 512      # 4 query blocks of 512
F16 = mybir.dt.float16
F32 = mybir.dt.float32
NEG = -1.0e30

_cache = {}


def _build():
    nc = bacc.Bacc("TRN2", target_bir_lowering=False, num_devices=8)

    xT = nc.dram_tensor("xT", [D, T], F16, kind="ExternalInput")
    wqk = nc.dram_tensor("wqk", [D, 2 * DL], F16, kind="ExternalInput")
    bqk = nc.dram_tensor("bqk", [128, 2 * DL // 128], F32, kind="ExternalInput")
    wv = nc.dram_tensor("wv", [D, DL], F16, kind="ExternalInput")
    wp = nc.dram_tensor("wp", [DL, D], F16, kind="ExternalInput")
    tri = nc.dram_tensor("tri", [128, 128], F32, kind="ExternalInput")
    out = nc.dram_tensor("out", [T, D], F16, kind="ExternalOutput")

    with tile.TileContext(nc) as tc:
        with (
            tc.tile_pool(name="const", bufs=1) as const,
            tc.tile_pool(name="acts", bufs=1) as acts,
            tc.tile_pool(name="work", bufs=4) as work,
            tc.tile_pool(name="small", bufs=4) as small,
            tc.tile_pool(name="outp", bufs=3) as outp,
            tc.tile_pool(name="psS", bufs=2, space="PSUM") as psS,
            tc.tile_pool(name="psY", bufs=2, space="PSUM") as psY,
            tc.tile_pool(name="psF", bufs=2, space="PSUM") as psF,
        ):
            # ---- SBUF residents ----
            xT_sb = [const.tile([128, T], F16, name=f"xT{k}", tag=f"xT{k}")
                     for k in range(KCH)]
            wqk_sb = [const.tile([128, 2 * DL], F16, name=f"wqk{k}",
                                 tag=f"wqk{k}") for k in range(KCH)]
            wv_sb = [const.tile([128, DL], F16, name=f"wv{k}", tag=f"wv{k}")
                     for k in range(KCH)]
            wp_sb = [const.tile([128, D], F16, name=f"wp{c}", tag=f"wp{c}")
                     for c in range(DL // 128)]
            bqk_sb = const.tile([128, 2 * DL // 128], F32)
            tri_sb = const.tile([128, 128], F32)

            qT_sb = [acts.tile([128, T], F16, name=f"qT{c}", tag=f"qT{c}")
                     for c in range(4)]
            # kT per head, zero-padded to K=128: head 2c in partitions 0:64
            # of kT2[2c], head 2c+1 in partitions 64:128 of kT2[2c+1]; the
            # other half stays zero so S matmuls run full-K (no row-group
            # masking -> PE clock stays at full rate).
            kT2_sb = [acts.tile([128, T], F16, name=f"kT2h{h}", tag=f"kT2h{h}")
                      for h in range(HL)]
            vaug = [acts.tile([128, HL * (HD + 1)], F16, name=f"va{t}",
                              tag=f"va{t}") for t in range(TCH)]
            yT_sb = [acts.tile([128, T], F16, name=f"yT{c}", tag=f"yT{c}")
                     for c in range(4)]

            # zero the unused kT halves before any S matmul reads them
            for h in range(HL):
                z0, z1 = (64, 128) if h % 2 == 0 else (0, 64)
                nc.gpsimd.memset(kT2_sb[h][z0:z1, :], 0.0)

            # ---- input DMAs on 4 queues, in consumption order ----
            # sync: xT block0 then block1; scalar: wqk (cc0/cc4 slices first);
            # vector: wv then xT blocks 2-3; gpsimd: bqk/tri/wp.
            for k in range(KCH):
                nc.sync.dma_start(out=xT_sb[k][:, 0:512],
                                  in_=xT[128 * k:128 * (k + 1), 0:512])
                nc.scalar.dma_start(out=wqk_sb[k][:, 0:128],
                                    in_=wqk[128 * k:128 * (k + 1), 0:128])
                nc.scalar.dma_start(out=wqk_sb[k][:, 512:640],
                                    in_=wqk[128 * k:128 * (k + 1), 512:640])
            for k in range(KCH):
                nc.gpsimd.dma_start(out=wv_sb[k],
                                    in_=wv[128 * k:128 * (k + 1), :])
                nc.sync.dma_start(out=xT_sb[k][:, 512:1024],
                                  in_=xT[128 * k:128 * (k + 1), 512:1024])
                nc.scalar.dma_start(out=wqk_sb[k][:, 128:512],
                                    in_=wqk[128 * k:128 * (k + 1), 128:512])
                nc.scalar.dma_start(out=wqk_sb[k][:, 640:1024],
                                    in_=wqk[128 * k:128 * (k + 1), 640:1024])
            nc.gpsimd.dma_start(out=bqk_sb, in_=bqk[:, :])
            nc.gpsimd.dma_start(out=tri_sb, in_=tri[:, :])
            for b in range(2, NB):
                for k in range(KCH):
                    nc.sync.dma_start(
                        out=xT_sb[k][:, 512 * b:512 * (b + 1)],
                        in_=xT[128 * k:128 * (k + 1), 512 * b:512 * (b + 1)])
            for c in range(DL // 128):
                nc.gpsimd.dma_start(out=wp_sb[c],
                                    in_=wp[128 * c:128 * (c + 1), :])

            # ---- filler step machinery ----
            fill = deque()

            def qk_chain_steps(cc, b):
                st = {}
                steps = []
                for k in range(KCH):
                    def mm(k=k, cc=cc, b=b):
                        if k == 0:
                            st["ps"] = psF.tile([128, 512], F32, name="psF",
                                                tag="psF")
                        nc.tensor.matmul(
                            st["ps"],
                            wqk_sb[k][:, 128 * cc:128 * (cc + 1)],
                            xT_sb[k][:, 512 * b:512 * (b + 1)],
                            start=(k == 0), stop=(k == KCH - 1))
                    steps.append(mm)

                def bias(cc=cc, b=b):
                    ps = st["ps"]
                    tbs = slice(512 * b, 512 * (b + 1))
                    if cc < 4:
                        nc.vector.tensor_scalar_add(
                            out=qT_sb[cc][:, tbs], in0=ps,
                            scalar1=bqk_sb[:, cc:cc + 1])
                    else:
                        hA = 2 * (cc - 4)
                        nc.vector.tensor_scalar_add(
                            out=kT2_sb[hA][0:64, tbs], in0=ps[0:64, :],
                            scalar1=bqk_sb[0:64, cc:cc + 1])
                        nc.vector.tensor_scalar_add(
                            out=kT2_sb[hA + 1][64:128, tbs],
                            in0=ps[64:128, :],
                            scalar1=bqk_sb[64:128, cc:cc + 1])
                steps.append(bias)
                return steps

            def v_chain_steps(t):
                st = {}
                steps = []
                for k in range(KCH):
                    def mm(k=k, t=t):
                        if k == 0:
                            st["ps"] = psF.tile([128, 512], F32, name="psF",
                                                tag="psF")
                        nc.tensor.matmul(
                            st["ps"],
                            xT_sb[k][:, 128 * t:128 * (t + 1)],
                            wv_sb[k],
                            start=(k == 0), stop=(k == KCH - 1))
                    steps.append(mm)

                def cast(t=t):
                    va3 = vaug[t].rearrange("p (h c) -> p h c", c=HD + 1)
                    nc.vector.tensor_copy(
                        va3[:, :, 0:HD],
                        st["ps"].rearrange("p (h d) -> p h d", d=HD))
                    nc.gpsimd.memset(va3[:, :, HD], 1.0)
                steps.append(cast)
                return steps

            def oproj_steps(t, half):
                st = {}
                steps = []
                for c in range(DL // 128):
                    def mm(c=c, t=t, half=half):
                        if c == 0:
                            st["ps"] = psF.tile([128, 512], F32, name="psF",
                                                tag="psF")
                        nc.tensor.matmul(
                            st["ps"],
                            yT_sb[c][:, 128 * t:128 * (t + 1)],
                            wp_sb[c][:, 512 * half:512 * (half + 1)],
                            start=(c == 0), stop=(c == DL // 128 - 1))
                    steps.append(mm)

                def castdma(t=t, half=half):
                    oc = outp.tile([128, 512], F16, name="oc", tag="oc")
                    nc.vector.tensor_copy(oc, st["ps"])
                    nc.gpsimd.dma_start(
                        out=out[128 * t:128 * (t + 1),
                                512 * half:512 * (half + 1)],
                        in_=oc)
                steps.append(castdma)
                return steps

            def qkv_block_steps(b):
                steps = []
                order = [0, 4]
                steps += qk_chain_steps(0, b)
                steps += qk_chain_steps(4, b)
                for t in range(4 * b, 4 * b + 4):
                    steps += v_chain_steps(t)
                for cc in (1, 5, 2, 6, 3, 7):
                    steps += qk_chain_steps(cc, b)
                return steps

            def oproj_block_steps(q0):
                steps = []
                for t in range(4 * q0, 4 * q0 + 4):
                    for half in range(2):
                        steps += oproj_steps(t, half)
                return steps

            # ---- QKV for block 0: emitted directly ----
            for st in qkv_block_steps(0):
                st()

            # ---- attention pipeline over query blocks ----
            for q0 in range(NB):
                if q0 + 1 < NB:
                    fill.extend(qkv_block_steps(q0 + 1))
                if q0 > 0:
                    fill.extend(oproj_block_steps(q0 - 1))

                ntiles = 4 * q0 + 4
                tiles_total = 4 * ntiles
                tiles_done = 0
                # block 0's filler (QKV of block 1) depends on late-arriving
                # x DMAs: keep its pace low so the PE never queues a stalled
                # filler matmul ahead of ready attention work.
                cap = 2 if q0 == 0 else 5
                for c in range(4):
                    ps_ys = [psY.tile([HD + 1, 512], F32, name="psY",
                                      tag="psY") for p in range(2)]
                    es_prev = None

                    def emit_PV(t, es):
                        m = t - 4 * q0
                        lo = 128 * m if m > 0 else 0
                        for p in range(2):
                            nc.tensor.matmul(
                                ps_ys[p][:, lo:512],
                                vaug[t][:, (HD + 1) * (2 * c + p):
                                        (HD + 1) * (2 * c + p + 1)],
                                es[:, 512 * p + lo:512 * (p + 1)],
                                start=(t == 0), stop=(t == ntiles - 1))

                    for t in range(ntiles):
                        m = t - 4 * q0
                        lo = 128 * m if m > 0 else 0
                        ps_s = psS.tile([128, 1024], F32, name="psS",
                                        tag="psS")
                        for p in range(2):
                            nc.tensor.matmul(
                                ps_s[:, 512 * p + lo:512 * (p + 1)],
                                kT2_sb[2 * c + p][:, 128 * t:128 * (t + 1)],
                                qT_sb[c][:, 512 * q0 + lo:512 * (q0 + 1)],
                                start=True, stop=True)
                        if m >= 0:
                            seg = ps_s.rearrange("p (u f) -> p u f", u=2)
                            nc.vector.tensor_add(
                                seg[:, :, lo:lo + 128],
                                seg[:, :, lo:lo + 128],
                                tri_sb.unsqueeze(1).broadcast_to(
                                    [128, 2, 128]))
                        es = work.tile([128, 1024], F16, name="es", tag="es")
                        nc.scalar.activation(
                            out=es[:, lo:1024], in_=ps_s[:, lo:1024],
                            func=mybir.ActivationFunctionType.Exp)
                        # filler between S(t) and PV(t-1): PE covers the
                        # exp latency with independent projection matmuls
                        left = tiles_total - tiles_done
                        pace = (len(fill) + left - 1) // left if fill else 0
                        for _ in range(min(pace, cap)):
                            if fill:
                                fill.popleft()()
                        tiles_done += 1
                        if es_prev is not None:
                            emit_PV(t - 1, es_prev)
                        es_prev = es
                    emit_PV(ntiles - 1, es_prev)

                    # normalize: y = yhat / denom (denom = ones-row of PV)
                    for p in range(2):
                        dn = small.tile([1, 512], F32, name="dn", tag="dn")
                        nc.vector.tensor_copy(dn, ps_ys[p][HD:HD + 1, :])
                        dnb = small.tile([64, 512], F32, name="dnb",
                                         tag="dnb")
                        nc.gpsimd.partition_broadcast(dnb, dn)
                        rcb = small.tile([64, 512], F32, name="rcb",
                                         tag="rcb")
                        nc.vector.reciprocal_approx_fast(rcb, dnb)
                        nc.vector.tensor_mul(
                            yT_sb[c][64 * p:64 * (p + 1),
                                     512 * q0:512 * (q0 + 1)],
                            ps_ys[p][0:HD, :],
                            rcb)

            # drain leftover filler, then the final block's projection
            while fill:
                fill.popleft()()
            for st in oproj_block_steps(NB - 1):
                st()

    nc.finalize()
    return nc


def _enable_trace_hooks():
    """Inject antenv.axon_hooks + no-op artifact upload so that
    run_bass_kernel_spmd(trace=True) works under axon in this image."""
    import types
    import antenv

    if "antenv.axon_hooks" not in sys.modules:
        mod = types.ModuleType("antenv.axon_hooks")
        state = {"hook": None}
        mod.set_axon_ntff_profile_hook = lambda h: state.__setitem__("hook", h)
        mod.get_axon_ntff_profile_hook = lambda: state["hook"]
        sys.modules["antenv.axon_hooks"] = mod
        antenv.axon_hooks = mod
        from trn_agent_boot.trn_boot import _ntff_profile_via_ctypes

        mod.set_axon_ntff_profile_hook(
            _ntff_profile_via_ctypes("/opt/axon/libaxon_pjrt.so"))
    from concourse import bass_utils as bu

    bu.upload_artifacts = lambda tmpdir: str(tmpdir)


def kernel(x, w_attn, b_attn, w_proj, b_proj, _trace=False):
    x = np.asarray(x)
    w_attn = np.asarray(w_attn)
    b_attn = np.asarray(b_attn)
    w_proj = np.asarray(w_proj)
    b_proj = np.asarray(b_proj)

    if "nc" not in _cache:
        _cache["nc"] = _build()
    nc = _cache["nc"]

    scale = 1.0 / np.sqrt(HD)
    f16 = np.float16
    tri = np.where(np.arange(128)[:, None] <= np.arange(128)[None, :],
                   np.float32(0.0), np.float32(NEG)).astype(np.float32)

    in_maps = []
    for core in range(8):
        b, hg = core // 2, core % 2
        qs = slice(hg * DL, (hg + 1) * DL)
        ks = slice(D + hg * DL, D + (hg + 1) * DL)
        wq = (w_attn[:, qs] * scale).astype(f16)
        wk = w_attn[:, ks].astype(f16)
        wqk_host = np.concatenate([wq, wk], axis=1)
        bqk_host = np.concatenate(
            [b_attn[qs] * scale, b_attn[ks]]).astype(np.float32)
        vs = slice(2 * D + hg * DL, 2 * D + (hg + 1) * DL)
        in_maps.append({
            "xT": np.ascontiguousarray(x[b].T).astype(f16),
            "wqk": np.ascontiguousarray(wqk_host),
            "bqk": np.ascontiguousarray(bqk_host.reshape(8, 128).T),
            "wv": np.ascontiguousarray(w_attn[:, vs]).astype(f16),
            "wp": np.ascontiguousarray(
                w_proj[hg * DL:(hg + 1) * DL, :]).astype(f16),
            "tri": tri,
        })

    kwargs = {}
    if _trace:
        _enable_trace_hooks()
        kwargs = dict(trace=True, trace_cores=[0])
    res = run_bass_kernel_spmd(nc, in_maps, core_ids=list(range(8)), **kwargs)

    # host epilogue: sum TP partials, add b_proj and the folded v-bias term
    bias_total = (b_attn[2 * D:].astype(np.float32) @
                  w_proj.astype(np.float32)) + b_proj.astype(np.float32)
    outp = np.empty((B, T, D), np.float32)
    for b in range(B):
        outp[b] = (res.results[2 * b]["out"].astype(np.float32) +
                   res.results[2 * b + 1]["out"].astype(np.float32))
    outp += bias_total

    if _trace:
        print(f"HW exec time: {res.exec_time_ns} ns")
    return outp


# revision 10
# speedup vs baseline: 1.1364x; 1.0324x over previous
"""Causal self-attention (B=4, T=2048, D=1024, H=16) on 8 trn2 NeuronCores.

Sharding: Megatron-style tensor parallel over heads (TP=2) x data parallel
over batch (DP=4). Core c handles batch c//2 and head-group c%2 (8 heads).
Each core computes its QKV projection slice, causal attention for its 8
heads, and a partial output projection; the host sums the two TP partials
per batch and adds b_proj (plus the folded v-bias contribution).

Schedule: single software-pipelined stream.  Attention for query block q0
is exp-rate-limited on the scalar engine, so independent projection
matmuls (QKV of block q0+1, output projection of earlier blocks) are
interleaved into the attention instruction stream as PE filler.  The v
bias is folded into a host-side output correction (attn rows sum to 1),
and the output partials are written as fp16 to halve the drain DMA.
"""
import sys
from collections import deque

sys.path.insert(0, "/opt/trn_rl_repo")

import numpy as np
import ml_dtypes

import concourse.bass as bass
import concourse.tile as tile
from concourse import bacc, mybir
from concourse.bass_utils import run_bass_kernel_spmd

B, T, D, H = 4, 2048, 1024, 16
HD = 64            # head dim
HL = 8             # heads per core (TP=2)
DL = HL * HD       # 512 local qkv width
KCH = D // 128     # 8 contraction chunks
TCH = T // 128     # 16 token tiles of 128
NB = T // 512      # 4 query blocks of 512
F16 = mybir.dt.float16
F32 = mybir.dt.float32
NEG = -1.0e30

_cache = {}


def _build():
    nc = bacc.Bacc("TRN2", target_bir_lowering=False, num_devices=8)

    xT = nc.dram_tensor("xT", [D, T], F16, kind="ExternalInput")
    wqk = nc.dram_tensor("wqk", [D, 2 * DL], F16, kind="ExternalInput")
    bqk = nc.dram_tensor("bqk", [128, 2 * DL // 128], F32, kind="ExternalInput")
    wv = nc.dram_tensor("wv", [D, DL], F16, kind="ExternalInput")
    wp = nc.dram_tensor("wp", [DL, D], F16, kind="ExternalInput")
    tri = nc.dram_tensor("tri", [128, 128], F32, kind="ExternalInput")
    out = nc.dram_tensor("out", [T, D], F16, kind="ExternalOutput")

    with tile.TileContext(nc) as tc:
        with (
            tc.tile_pool(name="const", bufs=1) as const,
            tc.tile_pool(name="acts", bufs=1) as acts,
            tc.tile_pool(name="work", bufs=4) as work,
            tc.tile_pool(name="small", bufs=4) as small,
            tc.tile_pool(name="outp", bufs=3) as outp,
            tc.tile_pool(name="psS", bufs=2, space="PSUM") as psS,
            tc.tile_pool(name="psY", bufs=2, space="PSUM") as psY,
            tc.tile_pool(name="psF", bufs=2, space="PSUM") as psF,
        ):
            # ---- SBUF residents ----
            xT_sb = [const.tile([128, T], F16, name=f"xT{k}", tag=f"xT{k}")
                     for k in range(KCH)]
            wqk_sb = [const.tile([128, 2 * DL], F16, name=f"wqk{k}",
                                 tag=f"wqk{k}") for k in range(KCH)]
            wv_sb = [const.tile([128, DL], F16, name=f"wv{k}", tag=f"wv{k}")
                     for k in range(KCH)]
            wp_sb = [const.tile([128, D], F16, name=f"wp{c}", tag=f"wp{c}")
                     for c in range(DL // 128)]
            bqk_sb = const.tile([128, 2 * DL // 128], F32)
            tri_sb = const.tile([128, 128], F32)

            qT_sb = [acts.tile([128, T], F16, name=f"qT{c}", tag=f"qT{c}")
                     for c in range(4)]
            # kT per head, zero-padded to K=128: head 2c in partitions 0:64
            # of kT2[2c], head 2c+1 in partitions 64:128 of kT2[2c+1]; the
            # other half stays zero so S matmuls run full-K (no row-group
            # masking -> PE clock stays at full rate).
            kT2_sb = [acts.tile([128, T], F16, name=f"kT2h{h}", tag=f"kT2h{h}")
                      for h in range(HL)]
            vaug = [acts.tile([128, HL * (HD + 1)], F16, name=f"va{t}",
                              tag=f"va{t}") for t in range(TCH)]
            yT_sb = [acts.tile([128, T], F16, name=f"yT{c}", tag=f"yT{c}")
                     for c in range(4)]

            # ---- input DMAs on 3 queues, in consumption order ----
            # sync: xT block0 then block1; scalar: wqk (cc0/cc4 slices first);
            # vector: wv then xT blocks 2-3; gpsimd: bqk/tri/wp.
            for k in range(KCH):
                nc.sync.dma_start(out=xT_sb[k][:, 0:512],
                                  in_=xT[128 * k:128 * (k + 1), 0:512])
                nc.scalar.dma_start(out=wqk_sb[k][:, 0:128],
                                    in_=wqk[128 * k:128 * (k + 1), 0:128])
                nc.scalar.dma_start(out=wqk_sb[k][:, 512:640],
                                    in_=wqk[128 * k:128 * (k + 1), 512:640])
            for k in range(KCH):
                nc.gpsimd.dma_start(out=wv_sb[k],
                                    in_=wv[128 * k:128 * (k + 1), :])
                nc.sync.dma_start(out=xT_sb[k][:, 512:1024],
                                  in_=xT[128 * k:128 * (k + 1), 512:1024])
                nc.scalar.dma_start(out=wqk_sb[k][:, 128:512],
                                    in_=wqk[128 * k:128 * (k + 1), 128:512])
                nc.scalar.dma_start(out=wqk_sb[k][:, 640:1024],
                                    in_=wqk[128 * k:128 * (k + 1), 640:1024])
            nc.gpsimd.dma_start(out=bqk_sb, in_=bqk[:, :])
            nc.gpsimd.dma_start(out=tri_sb, in_=tri[:, :])
            for b in range(2, NB):
                for k in range(KCH):
                    nc.sync.dma_start(
                        out=xT_sb[k][:, 512 * b:512 * (b + 1)],
                        in_=xT[128 * k:128 * (k + 1), 512 * b:512 * (b + 1)])
            for c in range(DL // 128):
                nc.gpsimd.dma_start(out=wp_sb[c],
                                    in_=wp[128 * c:128 * (c + 1), :])

            # zero the unused kT halves before any S matmul reads them
            # (after the DMA issues so they don't delay the gpsimd queue)
            for h in range(HL):
                z0, z1 = (64, 128) if h % 2 == 0 else (0, 64)
                nc.gpsimd.memset(kT2_sb[h][z0:z1, :], 0.0)

            # ---- filler step machinery ----
            fill = deque()

            def qk_chain_steps(cc, b):
                st = {}
                steps = []
                for k in range(KCH):
                    def mm(k=k, cc=cc, b=b):
                        if k == 0:
                            st["ps"] = psF.tile([128, 512], F32, name="psF",
                                                tag="psF")
                        nc.tensor.matmul(
                            st["ps"],
                            wqk_sb[k][:, 128 * cc:128 * (cc + 1)],
                            xT_sb[k][:, 512 * b:512 * (b + 1)],
                            start=(k == 0), stop=(k == KCH - 1))
                    steps.append(mm)

                def bias(cc=cc, b=b):
                    ps = st["ps"]
                    tbs = slice(512 * b, 512 * (b + 1))
                    if cc < 4:
                        nc.vector.tensor_scalar_add(
                            out=qT_sb[cc][:, tbs], in0=ps,
                            scalar1=bqk_sb[:, cc:cc + 1])
                    else:
                        hA = 2 * (cc - 4)
                        nc.vector.tensor_scalar_add(
                            out=kT2_sb[hA][0:64, tbs], in0=ps[0:64, :],
                            scalar1=bqk_sb[0:64, cc:cc + 1])
                        nc.vector.tensor_scalar_add(
                            out=kT2_sb[hA + 1][64:128, tbs],
                            in0=ps[64:128, :],
                            scalar1=bqk_sb[64:128, cc:cc + 1])
                steps.append(bias)
                return steps

            def v_chain_steps(t):
                st = {}
                steps = []
                for k in range(KCH):
                    def mm(k=k, t=t):
                        if k == 0:
                            st["ps"] = psF.tile([128, 512], F32, name="psF",
                                                tag="psF")
                        nc.tensor.matmul(
                            st["ps"],
                            xT_sb[k][:, 128 * t:128 * (t + 1)],
                            wv_sb[k],
                            start=(k == 0), stop=(k == KCH - 1))
                    steps.append(mm)

                def cast(t=t):
                    va3 = vaug[t].rearrange("p (h c) -> p h c", c=HD + 1)
                    nc.vector.tensor_copy(
                        va3[:, :, 0:HD],
                        st["ps"].rearrange("p (h d) -> p h d", d=HD))
                    nc.gpsimd.memset(va3[:, :, HD], 1.0)
                steps.append(cast)
                return steps

            def oproj_steps(t, half):
                st = {}
                steps = []
                for c in range(DL // 128):
                    def mm(c=c, t=t, half=half):
                        if c == 0:
                            st["ps"] = psF.tile([128, 512], F32, name="psF",
                                                tag="psF")
                        nc.tensor.matmul(
                            st["ps"],
                            yT_sb[c][:, 128 * t:128 * (t + 1)],
                            wp_sb[c][:, 512 * half:512 * (half + 1)],
                            start=(c == 0), stop=(c == DL // 128 - 1))
                    steps.append(mm)

                def castdma(t=t, half=half):
                    oc = outp.tile([128, 512], F16, name="oc", tag="oc")
                    nc.vector.tensor_copy(oc, st["ps"])
                    nc.gpsimd.dma_start(
                        out=out[128 * t:128 * (t + 1),
                                512 * half:512 * (half + 1)],
                        in_=oc)
                steps.append(castdma)
                return steps

            def qkv_block_steps(b):
                steps = []
                order = [0, 4]
                steps += qk_chain_steps(0, b)
                steps += qk_chain_steps(4, b)
                for t in range(4 * b, 4 * b + 4):
                    steps += v_chain_steps(t)
                for cc in (1, 5, 2, 6, 3, 7):
                    steps += qk_chain_steps(cc, b)
                return steps

            def oproj_block_steps(q0):
                steps = []
                for t in range(4 * q0, 4 * q0 + 4):
                    for half in range(2):
                        steps += oproj_steps(t, half)
                return steps

            # ---- QKV for block 0: emitted directly ----
            for st in qkv_block_steps(0):
                st()

            # ---- attention pipeline over query blocks ----
            for q0 in range(NB):
                if q0 + 1 < NB:
                    fill.extend(qkv_block_steps(q0 + 1))
                # spread out-projections so their DMAs aren't all at the
                # tail, but bias toward attention(3), whose exp load is
                # largest and needs the most PE filler.
                if q0 == 2:
                    fill.extend(oproj_block_steps(0))
                elif q0 == 3:
                    fill.extend(oproj_block_steps(1))
                    fill.extend(oproj_block_steps(2))

                ntiles = 4 * q0 + 4
                tiles_total = 4 * ntiles
                tiles_done = 0
                for c in range(4):
                    ps_ys = [psY.tile([HD + 1, 512], F32, name="psY",
                                      tag="psY") for p in range(2)]
                    pend = deque()   # PV pipeline, depth 2

                    def emit_PV(t, es):
                        m = t - 4 * q0
                        lo = 128 * m if m > 0 else 0
                        for p in range(2):
                            nc.tensor.matmul(
                                ps_ys[p][:, lo:512],
                                vaug[t][:, (HD + 1) * (2 * c + p):
                                        (HD + 1) * (2 * c + p + 1)],
                                es[:, 512 * p + lo:512 * (p + 1)],
                                start=(t == 0), stop=(t == ntiles - 1))

                    for t in range(ntiles):
                        m = t - 4 * q0
                        lo = 128 * m if m > 0 else 0
                        ps_s = psS.tile([128, 1024], F32, name="psS",
                                        tag="psS")
                        for p in range(2):
                            nc.tensor.matmul(
                                ps_s[:, 512 * p + lo:512 * (p + 1)],
                                kT2_sb[2 * c + p][:, 128 * t:128 * (t + 1)],
                                qT_sb[c][:, 512 * q0 + lo:512 * (q0 + 1)],
                                start=True, stop=True)
                        if m >= 0:
                            seg = ps_s.rearrange("p (u f) -> p u f", u=2)
                            nc.vector.tensor_add(
                                seg[:, :, lo:lo + 128],
                                seg[:, :, lo:lo + 128],
                                tri_sb.unsqueeze(1).broadcast_to(
                                    [128, 2, 128]))
                        es = work.tile([128, 1024], F16, name="es", tag="es")
                        nc.scalar.activation(
                            out=es[:, lo:1024], in_=ps_s[:, lo:1024],
                            func=mybir.ActivationFunctionType.Exp)
                        # filler between S(t) and PV(t-2): PE covers the
                        # exp latency with independent projection matmuls
                        left = tiles_total - tiles_done
                        pace = (len(fill) + left - 1) // left if fill else 0
                        for _ in range(pace):
                            if fill:
                                fill.popleft()()
                        tiles_done += 1
                        pend.append((t, es))
                        if len(pend) > 2:
                            emit_PV(*pend.popleft())
                    while pend:
                        emit_PV(*pend.popleft())

                    # normalize: y = yhat / denom (denom = ones-row of PV);
                    # p0/p1 interleaved so DVE and gpsimd pipeline
                    dn = [small.tile([1, 512], F32, name="dn", tag="dn")
                          for p in range(2)]
                    dnb = [small.tile([64, 512], F32, name="dnb", tag="dnb")
                           for p in range(2)]
                    rcb = [small.tile([64, 512], F32, name="rcb", tag="rcb")
                           for p in range(2)]
                    for p in range(2):
                        nc.vector.tensor_copy(dn[p], ps_ys[p][HD:HD + 1, :])
                    for p in range(2):
                        nc.gpsimd.partition_broadcast(dnb[p], dn[p])
                    for p in range(2):
                        nc.vector.reciprocal_approx_fast(rcb[p], dnb[p])
                    for p in range(2):
                        nc.vector.tensor_mul(
                            yT_sb[c][64 * p:64 * (p + 1),
                                     512 * q0:512 * (q0 + 1)],
                            ps_ys[p][0:HD, :],
                            rcb[p])

            # drain leftover filler, then the final block's projection
            while fill:
                fill.popleft()()
            for st in oproj_block_steps(NB - 1):
                st()

    nc.finalize()
    return nc


def _enable_trace_hooks():
    """Inject antenv.axon_hooks + no-op artifact upload so that
    run_bass_kernel_spmd(trace=True) works under axon in this image."""
    import types
    import antenv

    if "antenv.axon_hooks" not in sys.modules:
        mod = types.ModuleType("antenv.axon_hooks")
        state = {"hook": None}
        mod.set_axon_ntff_profile_hook = lambda h: state.__setitem__("hook", h)
        mod.get_axon_ntff_profile_hook = lambda: state["hook"]
        sys.modules["antenv.axon_hooks"] = mod
        antenv.axon_hooks = mod
        from trn_agent_boot.trn_boot import _ntff_profile_via_ctypes

        mod.set_axon_ntff_profile_hook(
            _ntff_profile_via_ctypes("/opt/axon/libaxon_pjrt.so"))
    from concourse import bass_utils as bu

    bu.upload_artifacts = lambda tmpdir: str(tmpdir)


def kernel(x, w_attn, b_attn, w_proj, b_proj, _trace=False):
    x = np.asarray(x)
    w_attn = np.asarray(w_attn)
    b_attn = np.asarray(b_attn)
    w_proj = np.asarray(w_proj)
    b_proj = np.asarray(b_proj)

    if "nc" not in _cache:
        _cache["nc"] = _build()
    nc = _cache["nc"]

    scale = 1.0 / np.sqrt(HD)
    f16 = np.float16
    tri = np.where(np.arange(128)[:, None] <= np.arange(128)[None, :],
                   np.float32(0.0), np.float32(NEG)).astype(np.float32)

    in_maps = []
    for core in range(8):
        b, hg = core // 2, core % 2
        qs = slice(hg * DL, (hg + 1) * DL)
        ks = slice(D + hg * DL, D + (hg + 1) * DL)
        wq = (w_attn[:, qs] * scale).astype(f16)
        wk = w_attn[:, ks].astype(f16)
        wqk_host = np.concatenate([wq, wk], axis=1)
        bqk_host = np.concatenate(
            [b_attn[qs] * scale, b_attn[ks]]).astype(np.float32)
        vs = slice(2 * D + hg * DL, 2 * D + (hg + 1) * DL)
        in_maps.append({
            "xT": np.ascontiguousarray(x[b].T).astype(f16),
            "wqk": np.ascontiguousarray(wqk_host),
            "bqk": np.ascontiguousarray(bqk_host.reshape(8, 128).T),
            "wv": np.ascontiguousarray(w_attn[:, vs]).astype(f16),
            "wp": np.ascontiguousarray(
                w_proj[hg * DL:(hg + 1) * DL, :]).astype(f16),
            "tri": tri,
        })

    kwargs = {}
    if _trace:
        _enable_trace_hooks()
        kwargs = dict(trace=True, trace_cores=[0])
    res = run_bass_kernel_spmd(nc, in_maps, core_ids=list(range(8)), **kwargs)

    # host epilogue: sum TP partials, add b_proj and the folded v-bias term
    bias_total = (b_attn[2 * D:].astype(np.float32) @
                  w_proj.astype(np.float32)) + b_proj.astype(np.float32)
    outp = np.empty((B, T, D), np.float32)
    for b in range(B):
        outp[b] = (res.results[2 * b]["out"].astype(np.float32) +
                   res.results[2 * b + 1]["out"].astype(np.float32))
    outp += bias_total

    if _trace:
        print(f"HW exec time: {res.exec_time_ns} ns")
    return outp


# revision 15
# speedup vs baseline: 1.2021x; 1.0578x over previous
"""Causal self-attention (B=4, T=2048, D=1024, H=16) on 8 trn2 NeuronCores.

Sharding: Megatron-style tensor parallel over heads (TP=2) x data parallel
over batch (DP=4). Core c handles batch c//2 and head-group c%2 (8 heads).
Each core computes its QKV projection slice, causal attention for its 8
heads, and a partial output projection; the host sums the two TP partials
per batch and adds b_proj (plus the folded v-bias contribution).

Schedule: single software-pipelined stream.  Attention for query block q0
is exp-rate-limited on the scalar engine, so independent projection
matmuls (QKV of block q0+1, output projection of earlier blocks) are
interleaved into the attention instruction stream as PE filler.  The v
bias is folded into a host-side output correction (attn rows sum to 1),
and the output partials are written as fp16 to halve the drain DMA.
"""
import sys
from collections import deque

sys.path.insert(0, "/opt/trn_rl_repo")

import numpy as np
import ml_dtypes

import concourse.bass as bass
import concourse.tile as tile
from concourse import bacc, mybir
from concourse.bass_utils import run_bass_kernel_spmd

B, T, D, H = 4, 2048, 1024, 16
HD = 64            # head dim
HL = 8             # heads per core (TP=2)
DL = HL * HD       # 512 local qkv width
KCH = D // 128     # 8 contraction chunks
TCH = T // 128     # 16 token tiles of 128
NB = T // 512      # 4 query blocks of 512
F16 = mybir.dt.float16
F32 = mybir.dt.float32
NEG = -1.0e30

_cache = {}


def _build():
    nc = bacc.Bacc("TRN2", target_bir_lowering=False, num_devices=8)

    xT = nc.dram_tensor("xT", [D, T], F16, kind="ExternalInput")
    wqk = nc.dram_tensor("wqk", [D, 2 * DL], F16, kind="ExternalInput")
    bqk = nc.dram_tensor("bqk", [128, 2 * DL // 128], F32, kind="ExternalInput")
    wv = nc.dram_tensor("wv", [D, DL], F16, kind="ExternalInput")
    wp = nc.dram_tensor("wp", [DL, D], F16, kind="ExternalInput")
    tri = nc.dram_tensor("tri", [128, 128], F32, kind="ExternalInput")
    out = nc.dram_tensor("out", [T, D], F16, kind="ExternalOutput")

    with tile.TileContext(nc) as tc:
        with (
            tc.tile_pool(name="const", bufs=1) as const,
            tc.tile_pool(name="acts", bufs=1) as acts,
            tc.tile_pool(name="work", bufs=4) as work,
            tc.tile_pool(name="small", bufs=4) as small,
            tc.tile_pool(name="outp", bufs=3) as outp,
            tc.tile_pool(name="psS", bufs=2, space="PSUM") as psS,
            tc.tile_pool(name="psY", bufs=2, space="PSUM") as psY,
            tc.tile_pool(name="psF", bufs=2, space="PSUM") as psF,
        ):
            # ---- SBUF residents ----
            xT_sb = [const.tile([128, T], F16, name=f"xT{k}", tag=f"xT{k}")
                     for k in range(KCH)]
            wqk_sb = [const.tile([128, 2 * DL], F16, name=f"wqk{k}",
                                 tag=f"wqk{k}") for k in range(KCH)]
            wv_sb = [const.tile([128, DL], F16, name=f"wv{k}", tag=f"wv{k}")
                     for k in range(KCH)]
            wp_sb = [const.tile([128, D], F16, name=f"wp{c}", tag=f"wp{c}")
                     for c in range(DL // 128)]
            bqk_sb = const.tile([128, 2 * DL // 128], F32)
            tri_sb = const.tile([128, 128], F32)

            qT_sb = [acts.tile([128, T], F16, name=f"qT{c}", tag=f"qT{c}")
                     for c in range(4)]
            # kT per head, zero-padded to K=128: head 2c in partitions 0:64
            # of kT2[2c], head 2c+1 in partitions 64:128 of kT2[2c+1]; the
            # other half stays zero so S matmuls run full-K (no row-group
            # masking -> PE clock stays at full rate).
            kT2_sb = [acts.tile([128, T], F16, name=f"kT2h{h}", tag=f"kT2h{h}")
                      for h in range(HL)]
            vaug = [acts.tile([128, HL * (HD + 1)], F16, name=f"va{t}",
                              tag=f"va{t}") for t in range(TCH)]
            yT_sb = [acts.tile([128, T], F16, name=f"yT{c}", tag=f"yT{c}")
                     for c in range(4)]

            # ---- input DMAs on 3 queues, in consumption order ----
            # The v chains of block 0 are the first consumers: they need
            # xT t-tile 0 (all k chunks) + wv, so those go first at fine
            # granularity.  sync: xT; scalar: wqk; gpsimd: wv/bqk/tri/wp.
            for t in range(4):
                for k in range(KCH):
                    nc.sync.dma_start(
                        out=xT_sb[k][:, 128 * t:128 * (t + 1)],
                        in_=xT[128 * k:128 * (k + 1), 128 * t:128 * (t + 1)])
            for k in range(KCH):
                nc.gpsimd.dma_start(out=wv_sb[k],
                                    in_=wv[128 * k:128 * (k + 1), :])
                nc.scalar.dma_start(out=wqk_sb[k][:, 0:128],
                                    in_=wqk[128 * k:128 * (k + 1), 0:128])
                nc.scalar.dma_start(out=wqk_sb[k][:, 512:640],
                                    in_=wqk[128 * k:128 * (k + 1), 512:640])
            nc.gpsimd.dma_start(out=bqk_sb, in_=bqk[:, :])
            nc.gpsimd.dma_start(out=tri_sb, in_=tri[:, :])
            for k in range(KCH):
                nc.sync.dma_start(out=xT_sb[k][:, 512:1024],
                                  in_=xT[128 * k:128 * (k + 1), 512:1024])
                nc.scalar.dma_start(out=wqk_sb[k][:, 128:512],
                                    in_=wqk[128 * k:128 * (k + 1), 128:512])
                nc.scalar.dma_start(out=wqk_sb[k][:, 640:1024],
                                    in_=wqk[128 * k:128 * (k + 1), 640:1024])
            for b in range(2, NB):
                for k in range(KCH):
                    nc.sync.dma_start(
                        out=xT_sb[k][:, 512 * b:512 * (b + 1)],
                        in_=xT[128 * k:128 * (k + 1), 512 * b:512 * (b + 1)])
            for c in range(DL // 128):
                nc.gpsimd.dma_start(out=wp_sb[c],
                                    in_=wp[128 * c:128 * (c + 1), :])

            # zero the unused kT halves before any S matmul reads them
            # (after the DMA issues so they don't delay the gpsimd queue)
            for h in range(HL):
                z0, z1 = (64, 128) if h % 2 == 0 else (0, 64)
                nc.gpsimd.memset(kT2_sb[h][z0:z1, :], 0.0)

            # ---- filler step machinery ----
            fill = deque()

            def qk_chain_steps(cc, b):
                st = {}
                steps = []
                for k in range(KCH):
                    def mm(k=k, cc=cc, b=b):
                        if k == 0:
                            st["ps"] = psF.tile([128, 512], F32, name="psF",
                                                tag="psF")
                        nc.tensor.matmul(
                            st["ps"],
                            wqk_sb[k][:, 128 * cc:128 * (cc + 1)],
                            xT_sb[k][:, 512 * b:512 * (b + 1)],
                            start=(k == 0), stop=(k == KCH - 1))
                    steps.append(mm)

                def bias(cc=cc, b=b):
                    ps = st["ps"]
                    tbs = slice(512 * b, 512 * (b + 1))
                    if cc < 4:
                        nc.vector.tensor_scalar_add(
                            out=qT_sb[cc][:, tbs], in0=ps,
                            scalar1=bqk_sb[:, cc:cc + 1])
                    else:
                        hA = 2 * (cc - 4)
                        nc.vector.tensor_scalar_add(
                            out=kT2_sb[hA][0:64, tbs], in0=ps[0:64, :],
                            scalar1=bqk_sb[0:64, cc:cc + 1])
                        nc.vector.tensor_scalar_add(
                            out=kT2_sb[hA + 1][64:128, tbs],
                            in0=ps[64:128, :],
                            scalar1=bqk_sb[64:128, cc:cc + 1])
                steps.append(bias)
                return steps

            def v_chain_steps(t):
                st = {}
                steps = []
                for k in range(KCH):
                    def mm(k=k, t=t):
                        if k == 0:
                            st["ps"] = psF.tile([128, 512], F32, name="psF",
                                                tag="psF")
                        nc.tensor.matmul(
                            st["ps"],
                            xT_sb[k][:, 128 * t:128 * (t + 1)],
                            wv_sb[k],
                            start=(k == 0), stop=(k == KCH - 1))
                    steps.append(mm)

                def cast(t=t):
                    va3 = vaug[t].rearrange("p (h c) -> p h c", c=HD + 1)
                    nc.vector.tensor_copy(
                        va3[:, :, 0:HD],
                        st["ps"].rearrange("p (h d) -> p h d", d=HD))
                    nc.gpsimd.memset(va3[:, :, HD], 1.0)
                steps.append(cast)
                return steps

            def oproj_steps(t, half):
                st = {}
                steps = []
                for c in range(DL // 128):
                    def mm(c=c, t=t, half=half):
                        if c == 0:
                            st["ps"] = psF.tile([128, 512], F32, name="psF",
                                                tag="psF")
                        nc.tensor.matmul(
                            st["ps"],
                            yT_sb[c][:, 128 * t:128 * (t + 1)],
                            wp_sb[c][:, 512 * half:512 * (half + 1)],
                            start=(c == 0), stop=(c == DL // 128 - 1))
                    steps.append(mm)

                def castdma(t=t, half=half):
                    oc = outp.tile([128, 512], F16, name="oc", tag="oc")
                    nc.vector.tensor_copy(oc, st["ps"])
                    eng = (nc.sync, nc.scalar, nc.gpsimd)[(2 * t + half) % 3]
                    eng.dma_start(
                        out=out[128 * t:128 * (t + 1),
                                512 * half:512 * (half + 1)],
                        in_=oc)
                steps.append(castdma)
                return steps

            def qkv_block_steps(b):
                steps = []
                steps += qk_chain_steps(0, b)
                steps += qk_chain_steps(4, b)
                for t in range(4 * b, 4 * b + 4):
                    steps += v_chain_steps(t)
                for cc in (1, 5, 2, 6, 3, 7):
                    steps += qk_chain_steps(cc, b)
                return steps

            def oproj_block_steps(q0):
                steps = []
                for t in range(4 * q0, 4 * q0 + 4):
                    for half in range(2):
                        steps += oproj_steps(t, half)
                return steps

            fill_pushed = [0]
            fill_popped = [0]

            def push(steps):
                fill.extend(steps)
                fill_pushed[0] += len(steps)
                return fill_pushed[0]

            def pop_fill(n):
                for _ in range(n):
                    if fill:
                        fill.popleft()()
                        fill_popped[0] += 1

            def drain_to(mark):
                while fill_popped[0] < mark and fill:
                    fill.popleft()()
                    fill_popped[0] += 1

            # ---- QKV for block 0: v + first head-pair only, the rest is
            # filler so attention(0) starts as early as possible ----
            for t in range(4):
                for st in v_chain_steps(t):
                    st()
            for st in qk_chain_steps(0, 0) + qk_chain_steps(4, 0):
                st()
            b0_marks = {}
            for cgrp, ccs in enumerate([(1, 5), (2, 6), (3, 7)]):
                for cc in ccs:
                    push(qk_chain_steps(cc, 0))
                b0_marks[cgrp + 1] = fill_pushed[0]

            # ---- attention pipeline over query blocks ----
            prev_qkv_mark = 0
            for q0 in range(NB):
                # all of block q0's QKV must be emitted before its attention
                drain_to(prev_qkv_mark)
                if q0 + 1 < NB:
                    prev_qkv_mark = push(qkv_block_steps(q0 + 1))
                # all out-projections run as filler inside attention(3),
                # whose exp load is largest; their DMAs round-robin the
                # three queues so the drain is spread out.
                if q0 == 3:
                    for qq in range(3):
                        push(oproj_block_steps(qq))

                ntiles = 4 * q0 + 4
                tiles_total = 4 * ntiles
                tiles_done = 0
                for c in range(4):
                    if q0 == 0 and c > 0:
                        drain_to(b0_marks[c])
                    ps_ys = [psY.tile([HD + 1, 512], F32, name="psY",
                                      tag="psY") for p in range(2)]
                    pend = deque()   # PV pipeline, depth 2

                    def emit_PV(t, es):
                        m = t - 4 * q0
                        lo = 128 * m if m > 0 else 0
                        for p in range(2):
                            nc.tensor.matmul(
                                ps_ys[p][:, lo:512],
                                vaug[t][:, (HD + 1) * (2 * c + p):
                                        (HD + 1) * (2 * c + p + 1)],
                                es[:, 512 * p + lo:512 * (p + 1)],
                                start=(t == 0), stop=(t == ntiles - 1))

                    for t in range(ntiles):
                        m = t - 4 * q0
                        lo = 128 * m if m > 0 else 0
                        ps_s = psS.tile([128, 1024], F32, name="psS",
                                        tag="psS")
                        for p in range(2):
                            nc.tensor.matmul(
                                ps_s[:, 512 * p + lo:512 * (p + 1)],
                                kT2_sb[2 * c + p][:, 128 * t:128 * (t + 1)],
                                qT_sb[c][:, 512 * q0 + lo:512 * (q0 + 1)],
                                start=True, stop=True)
                        if m >= 0:
                            seg = ps_s.rearrange("p (u f) -> p u f", u=2)
                            nc.vector.tensor_add(
                                seg[:, :, lo:lo + 128],
                                seg[:, :, lo:lo + 128],
                                tri_sb.unsqueeze(1).broadcast_to(
                                    [128, 2, 128]))
                        es = work.tile([128, 1024], F16, name="es", tag="es")
                        nc.scalar.activation(
                            out=es[:, lo:1024], in_=ps_s[:, lo:1024],
                            func=mybir.ActivationFunctionType.Exp)
                        # filler between S(t) and PV(t-3): PE covers the
                        # exp latency with independent projection matmuls
                        left = tiles_total - tiles_done
                        pace = (len(fill) + left - 1) // left if fill else 0
                        pop_fill(pace)
                        tiles_done += 1
                        pend.append((t, es))
                        if len(pend) > 3:
                            emit_PV(*pend.popleft())
                    while pend:
                        emit_PV(*pend.popleft())

                    # normalize: y = yhat / denom (denom = ones-row of PV);
                    # p0/p1 interleaved so DVE and gpsimd pipeline
                    dn = [small.tile([1, 512], F32, name="dn", tag="dn")
                          for p in range(2)]
                    dnb = [small.tile([64, 512], F32, name="dnb", tag="dnb")
                           for p in range(2)]
                    rcb = [small.tile([64, 512], F32, name="rcb", tag="rcb")
                           for p in range(2)]
                    for p in range(2):
                        nc.vector.tensor_copy(dn[p], ps_ys[p][HD:HD + 1, :])
                    for p in range(2):
                        nc.gpsimd.partition_broadcast(dnb[p], dn[p])
                    for p in range(2):
                        nc.vector.reciprocal_approx_fast(rcb[p], dnb[p])
                    for p in range(2):
                        nc.vector.tensor_mul(
                            yT_sb[c][64 * p:64 * (p + 1),
                                     512 * q0:512 * (q0 + 1)],
                            ps_ys[p][0:HD, :],
                            rcb[p])

            # drain leftover filler, then the final block's projection
            while fill:
                fill.popleft()()
            for st in oproj_block_steps(NB - 1):
                st()

    nc.finalize()
    return nc


def _enable_trace_hooks():
    """Inject antenv.axon_hooks + no-op artifact upload so that
    run_bass_kernel_spmd(trace=True) works under axon in this image."""
    import types
    import antenv

    if "antenv.axon_hooks" not in sys.modules:
        mod = types.ModuleType("antenv.axon_hooks")
        state = {"hook": None}
        mod.set_axon_ntff_profile_hook = lambda h: state.__setitem__("hook", h)
        mod.get_axon_ntff_profile_hook = lambda: state["hook"]
        sys.modules["antenv.axon_hooks"] = mod
        antenv.axon_hooks = mod
        from trn_agent_boot.trn_boot import _ntff_profile_via_ctypes

        mod.set_axon_ntff_profile_hook(
            _ntff_profile_via_ctypes("/opt/axon/libaxon_pjrt.so"))
    from concourse import bass_utils as bu

    bu.upload_artifacts = lambda tmpdir: str(tmpdir)


def kernel(x, w_attn, b_attn, w_proj, b_proj, _trace=False):
    x = np.asarray(x)
    w_attn = np.asarray(w_attn)
    b_attn = np.asarray(b_attn)
    w_proj = np.asarray(w_proj)
    b_proj = np.asarray(b_proj)

    if "nc" not in _cache:
        _cache["nc"] = _build()
    nc = _cache["nc"]

    scale = 1.0 / np.sqrt(HD)
    f16 = np.float16
    tri = np.where(np.arange(128)[:, None] <= np.arange(128)[None, :],
                   np.float32(0.0), np.float32(NEG)).astype(np.float32)

    in_maps = []
    for core in range(8):
        b, hg = core // 2, core % 2
        qs = slice(hg * DL, (hg + 1) * DL)
        ks = slice(D + hg * DL, D + (hg + 1) * DL)
        wq = (w_attn[:, qs] * scale).astype(f16)
        wk = w_attn[:, ks].astype(f16)
        wqk_host = np.concatenate([wq, wk], axis=1)
        bqk_host = np.concatenate(
            [b_attn[qs] * scale, b_attn[ks]]).astype(np.float32)
        vs = slice(2 * D + hg * DL, 2 * D + (hg + 1) * DL)
        in_maps.append({
            "xT": np.ascontiguousarray(x[b].T).astype(f16),
            "wqk": np.ascontiguousarray(wqk_host),
            "bqk": np.ascontiguousarray(bqk_host.reshape(8, 128).T),
            "wv": np.ascontiguousarray(w_attn[:, vs]).astype(f16),
            "wp": np.ascontiguousarray(
                w_proj[hg * DL:(hg + 1) * DL, :]).astype(f16),
            "tri": tri,
        })

    kwargs = {}
    if _trace:
        _enable_trace_hooks()
        kwargs = dict(trace=True, trace_cores=[0])
    res = run_bass_kernel_spmd(nc, in_maps, core_ids=list(range(8)), **kwargs)

    # host epilogue: sum TP partials, add b_proj and the folded v-bias term
    bias_total = (b_attn[2 * D:].astype(np.float32) @
                  w_proj.astype(np.float32)) + b_proj.astype(np.float32)
    outp = np.empty((B, T, D), np.float32)
    for b in range(B):
        outp[b] = (res.results[2 * b]["out"].astype(np.float32) +
                   res.results[2 * b + 1]["out"].astype(np.float32))
    outp += bias_total

    if _trace:
        print(f"HW exec time: {res.exec_time_ns} ns")
    return outp
